# revision 1
# baseline (speedup 1.0000x reference)
"""Distributed Bass kernel for nn_AttentionLayer (2-branch GAT-style layer).

Row-shard over 8 NeuronCores (512 rows each). All per-row tensors are kept
in "transposed" layout on chip (feature/column axis on SBUF partitions) so
that the masked softmax feeds the PE attention matmuls without transposes:

  e_b^T[k, i] = lrelu(s1_b[i] + s2_b[k])                (k on partitions)
  z = e + (mask01 - 1)*BIG ; p = exp(z)                 (exp underflow -> exact 0)
  out_b^T[f, i] = sum_k Wh_b[k, f] * p[k, i]            (PE, bf16)

adj2^T is computed on PE in fp8 DoubleRow (exact: adj is 0/1, psum f32):
  adj2^T[k, i] = sum_t adj_full[t, k] * adjT_shard[t, i]

The adj AllGather is split into 8 column chunks so branch-2 groups pipeline
against the collective. BatchNorm batch stats via a tiny AllReduce. No
row-max subtraction in softmax (values are small, no empty rows).
"""

import sys
import numpy as np

for _p in ("/opt/trn_rl_repo", "/opt/trn_rl_repo/concourse"):
    if _p not in sys.path:
        sys.path.insert(0, _p)

import ml_dtypes

N = 4096
M_CORES = 8
R = N // M_CORES          # 512 rows per core
IN_F = 512
HALF = IN_F // 2          # 256
F = 64
P = 128                   # partitions
NT = N // P               # 32 column tiles
NTP = NT // 2             # 16 row-pair tiles (DoubleRow)
RT = R // P               # 4 row tiles per core
G = 4                     # adj2 k-tiles per psum group
NG = NT // G              # 8 groups == adj AG chunks
ALPHA = 0.2
EPS = 1e-5
BIG = 9e15
INV_N = 1.0 / N

_CACHED = {}


def build_nc():
    from concourse import bacc, tile, mybir

    f32 = mybir.dt.float32
    bf16 = mybir.dt.bfloat16
    fp8 = mybir.dt.float8e4
    Alu = mybir.AluOpType
    Act = mybir.ActivationFunctionType
    DR = mybir.MatmulPerfMode.DoubleRow

    nc = bacc.Bacc("TRN2", target_bir_lowering=False, debug=False,
                   num_devices=M_CORES)

    hT_p = nc.declare_dram_parameter("hT", [IN_F, R], f32, isOutput=False)
    hTf_p = nc.declare_dram_parameter("hTf", [IN_F, N], f32, isOutput=False)
    adjb8_p = nc.declare_dram_parameter("adjb8", [R, N], fp8, isOutput=False)
    adjbT_p = nc.declare_dram_parameter("adjbT", [N, R], fp8, isOutput=False)
    dT_p = nc.declare_dram_parameter("dT", [N, R], bf16, isOutput=False)
    W_p = nc.declare_dram_parameter("W12", [HALF, 2 * F], f32, isOutput=False)
    a_p = nc.declare_dram_parameter("a", [F, 2], f32, isOutput=False)
    gb_p = nc.declare_dram_parameter("gb", [2 * F, 2], f32, isOutput=False)
    id_p = nc.declare_dram_parameter("ident", [P, P], f32, isOutput=False)
    out_p = nc.declare_dram_parameter("out", [R, 2 * F], f32, isOutput=True)

    RG = [list(range(M_CORES))]

    with tile.TileContext(nc) as tc:
        with (
            tc.tile_pool(name="sb", bufs=1) as sb,
            tc.tile_pool(name="sbt", bufs=3) as sbt,
            tc.tile_pool(name="psA", bufs=1, space="PSUM") as psA,
            tc.tile_pool(name="psB", bufs=5, space="PSUM") as psB,
            tc.tile_pool(name="dram", bufs=1, space="DRAM") as dram,
        ):
            # ---- small persistent loads (sync queue; gate the Wh compute)
            ident = sb.tile([P, P], f32)
            nc.sync.dma_start(ident[:], id_p[:])
            a_sb = sb.tile([F, 2], f32)
            nc.sync.dma_start(a_sb[:], a_p[:])
            gb_sb = sb.tile([2 * F, 2], f32)
            nc.sync.dma_start(gb_sb[:], gb_p[:])
            W_sb = []
            for t in range(2):
                w = sb.tile([P, 2 * F], f32, tag=f"w{t}")
                nc.sync.dma_start(w[:], W_p[P * t:P * (t + 1), :])
                W_sb.append(w)
            hT_sb = []
            for t in range(RT):
                ht = sb.tile([P, R], f32, tag=f"ht{t}")
                nc.sync.dma_start(ht[:], hT_p[P * t:P * (t + 1), :])
                hT_sb.append(ht)

            ones1 = sb.tile([1, P], f32)
            nc.vector.memset(ones1[:], 1.0)
            onesb = sb.tile([P, 1], bf16)
            nc.vector.memset(onesb[:], 1.0)

            # ---- adj fp8 chunk bounces for chunked AllGather (gpsimd queue)
            adj_in = []
            for c in range(NG):
                ai = dram.tile([R, R], fp8, name=f"adj_in{c}")
                nc.gpsimd.dma_start(ai[:], adjb8_p[:, R * c:R * (c + 1)])
                adj_in.append(ai)

            # ---- Wh^T = W^T @ h^T  (psum [128, 512]: b1 rows 0:64, b2 64:128)
            whT_ps = psA.tile([P, R], f32, tag="acc")
            for b in range(2):
                for t in range(2):
                    nc.tensor.matmul(
                        whT_ps[F * b:F * (b + 1), :],
                        W_sb[t][:, F * b:F * (b + 1)],
                        hT_sb[2 * b + t][:],
                        start=(t == 0), stop=(t == 1),
                    )
            whT_sb = sb.tile([P, R], f32)
            nc.vector.tensor_copy(whT_sb[:], whT_ps[:])
            # base-partition-0 copy of Wh2^T (PE shift via identity)
            wh2_ps = psB.tile([F, R], f32, tag="tmp")
            nc.tensor.matmul(wh2_ps[:], ident[F:P, F:P], whT_sb[F:P, :],
                             start=True, stop=True)
            whT2_sb = sb.tile([F, R], f32)
            nc.vector.tensor_copy(whT2_sb[:], wh2_ps[:])
            whT_b = [whT_sb, whT2_sb]

            # ---- s1 vectors (own rows): s1_b[i] = sum_f a1[f] * WhT_b[f, i]
            s1_sb = []
            for b in range(2):
                sv = psB.tile([1, R], f32, tag="tmp")
                nc.tensor.matmul(sv[:], a_sb[:, 0:1], whT_b[b][0:F, :],
                                 start=True, stop=True)
                dst = sb.tile([1, R], f32, tag=f"s1_{b}")
                nc.vector.tensor_copy(dst[:], sv[:])
                s1_sb.append(dst)

            # ---- collectives: only the 8 adj chunk AllGathers (+ final AR)
            adj_chunk = []
            for c in range(NG):
                ac = dram.tile([N, R], fp8, addr_space="Shared",
                               name=f"adj_chunk{c}")
                nc.gpsimd.collective_compute(
                    "AllGather", Alu.bypass, replica_groups=RG,
                    ins=[adj_in[c][:].opt()], outs=[ac[:].opt()])
                adj_chunk.append(ac)

            # ---- transposed adj shard (fp8, DoubleRow pairing) ----
            adjT_sb = []
            for t in range(NTP):
                at = sb.tile([P, 2, R], fp8, tag=f"adjT{t}")
                src = adjbT_p[2 * P * t:2 * P * (t + 1), :]
                nc.sync.dma_start(at[:], src.rearrange("(s p) i -> p s i",
                                                       p=P))
                adjT_sb.append(at)

            # ---- full Wh^T computed locally (no AllGather) ----
            whTf1 = sb.tile([F, N], f32)
            whTf2 = sb.tile([F, N], f32)
            whTf_b = [whTf1, whTf2]
            for ch in range(NG):
                hfs = []
                for t in range(RT):
                    hf = sbt.tile([P, R], f32, tag="hf", bufs=8)
                    nc.sync.dma_start(
                        hf[:], hTf_p[P * t:P * (t + 1), R * ch:R * (ch + 1)])
                    hfs.append(hf)
                for b in range(2):
                    wf_ps = psB.tile([F, R], f32, tag="tmp",
                                     name=f"wfps{ch}_{b}")
                    for t in range(2):
                        nc.tensor.matmul(
                            wf_ps[:],
                            W_sb[t][:, F * b:F * (b + 1)],
                            hfs[2 * b + t][:],
                            start=(t == 0), stop=(t == 1),
                        )
                    nc.vector.tensor_copy(
                        whTf_b[b][:, R * ch:R * (ch + 1)], wf_ps[:])

            # ---- s2 full, locally; spread to per-partition [p, kt] layout
            s2d = dram.tile([2, N], f32)
            for b in range(2):
                s2fl = sb.tile([1, N], f32, tag=f"s2fl{b}")
                for ch in range(NG):
                    sv = psB.tile([1, R], f32, tag="tmp")
                    nc.tensor.matmul(
                        sv[:], a_sb[:, 1:2],
                        whTf_b[b][:, R * ch:R * (ch + 1)],
                        start=True, stop=True)
                    nc.vector.tensor_copy(s2fl[:, R * ch:R * (ch + 1)], sv[:])
                nc.sync.dma_start(s2d[b:b + 1, :], s2fl[:])
            s2_sb = []
            for b in range(2):
                s2b = sb.tile([P, NT], f32, tag=f"s2_{b}")
                nc.sync.dma_start(s2b[:],
                                  s2d[b].rearrange("(kt p) -> p kt", p=P))
                s2_sb.append(s2b)

            # ---- Wh natural tiles [k, 2F] bf16 via local PE transpose ----
            whf_sb = []
            for t in range(NT):
                wf = sbt.tile([P, 2 * F], bf16, tag=f"whf{t}", bufs=1)
                for b in range(2):
                    tpw = psB.tile([P, F], f32, tag="tmp",
                                   name=f"tpw{t}_{b}")
                    nc.tensor.transpose(tpw[:],
                                        whTf_b[b][:, P * t:P * (t + 1)],
                                        ident[0:F, 0:F])
                    nc.vector.tensor_copy(wf[:, F * b:F * (b + 1)], tpw[:])
                whf_sb.append(wf)

            # ---- s1 broadcast across partitions (PE outer-product with ones)
            s1bc = []
            for b in range(2):
                bc = psB.tile([P, R], f32, tag="tmp")
                nc.tensor.matmul(bc[:], ones1[:], s1_sb[b][:],
                                 start=True, stop=True)
                s1b = sb.tile([P, R], f32, tag=f"s1bc{b}")
                nc.vector.tensor_copy(s1b[:], bc[:])
                s1bc.append(s1b)

            # ---- accumulators ----
            accT = psA.tile([P, R], f32, tag="acc")     # [0:64] b1, [64:128] b2
            sum_1 = psA.tile([1, R], f32, tag="sum1", name="sum_1")
            sum_2 = psA.tile([1, R], f32, tag="sum2", name="sum_2")
            sums = [sum_1, sum_2]

            def softmax_tile(b, kt, mask_done_ap):
                """mask_done_ap: f32 [P, R] with (mask01-1) in {-1, 0}."""
                u = sbt.tile([P, R], f32, tag="u")
                nc.scalar.activation(u[:], s1bc[b][:], Act.Identity,
                                     bias=s2_sb[b][:, kt:kt + 1])
                e = sbt.tile([P, R], f32, tag="e")
                nc.vector.scalar_tensor_tensor(
                    e[:], u[:], ALPHA, u[:], op0=Alu.mult, op1=Alu.max)
                z = sbt.tile([P, R], f32, tag="z")
                nc.vector.scalar_tensor_tensor(
                    z[:], mask_done_ap, BIG, e[:], op0=Alu.mult, op1=Alu.add)
                pt = sbt.tile([P, R], bf16, tag="pt", bufs=8)
                nc.scalar.activation(pt[:], z[:], Act.Exp)
                nc.tensor.matmul(sums[b][:], onesb[:], pt[:],
                                 start=(kt == 0), stop=(kt == NT - 1))
                nc.tensor.matmul(accT[F * b:F * (b + 1), :],
                                 whf_sb[kt][:, F * b:F * (b + 1)], pt[:],
                                 start=(kt == 0), stop=(kt == NT - 1))

            # ---- branches interleaved per adj chunk: branch-2 group g
            # (DoubleRow adj2 + softmax), then branch-1 tiles 4g..4g+3 ----
            for g in range(NG):
                cnts = [psB.tile([P, R], f32, tag="tmp", name=f"cnt{g}_{j}")
                        for j in range(G)]
                for t in range(NTP):
                    af = sbt.tile([P, 2, R], fp8, tag="af")
                    src = adj_chunk[g][2 * P * t:2 * P * (t + 1), :]
                    nc.sync.dma_start(af[:],
                                      src.rearrange("(s p) k -> p s k", p=P))
                    for j in range(G):
                        nc.tensor.matmul(cnts[j][:],
                                         af[:, :, P * j:P * (j + 1)],
                                         adjT_sb[t][:],
                                         perf_mode=DR,
                                         start=(t == 0), stop=(t == NTP - 1))
                for j in range(G):
                    kt = G * g + j
                    dt_t = sbt.tile([P, R], bf16, tag="dt")
                    nc.sync.dma_start(dt_t[:], dT_p[P * kt:P * (kt + 1), :])
                    m2 = sbt.tile([P, R], f32, tag="m")
                    nc.vector.tensor_scalar(m2[:], cnts[j][:], 1.0, -1.0,
                                            op0=Alu.min, op1=Alu.add)
                    nc.vector.tensor_tensor(m2[:], m2[:], dt_t[:],
                                            op=Alu.subtract)
                    softmax_tile(1, kt, m2[:])
                for j in range(G):
                    kt = G * g + j
                    m1 = sbt.tile([P, R], f32, tag="m")
                    nc.vector.tensor_scalar(m1[:],
                                            adjT_sb[kt // 2][:, kt % 2, :],
                                            -1.0, None, op0=Alu.add)
                    softmax_tile(0, kt, m1[:])

            # ---- epilogue: normalize, BN stats + AllReduce, BN, lrelu ----
            hpT = sb.tile([P, R], f32)
            for b in range(2):
                rc = sb.tile([1, R], f32, tag=f"rc{b}")
                nc.vector.reciprocal(rc[:], sums[b][:])
                bc = psB.tile([P, R], f32, tag="tmp")
                nc.tensor.matmul(bc[:], ones1[:], rc[:],
                                 start=True, stop=True)
                rb = sbt.tile([P, R], f32, tag="u")
                nc.vector.tensor_copy(rb[:], bc[:])
                nc.vector.tensor_mul(hpT[F * b:F * (b + 1), :],
                                     accT[F * b:F * (b + 1), :],
                                     rb[F * b:F * (b + 1), :])

            sx = sb.tile([2 * F, 2], f32)
            nc.vector.tensor_reduce(sx[:, 0:1], hpT[:],
                                    axis=mybir.AxisListType.X, op=Alu.add)
            scr = sbt.tile([P, R], bf16, tag="pt", bufs=8)
            nc.scalar.activation(scr[:], hpT[:], Act.Square,
                                 accum_out=sx[:, 1:2])
            stats_in = dram.tile([2 * F, 2], f32)
            nc.sync.dma_start(stats_in[:], sx[:])
            stats_out = dram.tile([2 * F, 2], f32, addr_space="Shared")
            nc.gpsimd.collective_compute(
                "AllReduce", Alu.add, replica_groups=RG,
                ins=[stats_in[:].opt()], outs=[stats_out[:].opt()])
            gst = sb.tile([2 * F, 2], f32)
            nc.sync.dma_start(gst[:], stats_out[:])

            mean = sb.tile([2 * F, 1], f32)
            nc.scalar.mul(mean[:], gst[:, 0:1], INV_N)
            ex2 = sb.tile([2 * F, 1], f32)
            nc.scalar.mul(ex2[:], gst[:, 1:2], INV_N)
            var = sb.tile([2 * F, 1], f32)
            nc.vector.scalar_tensor_tensor(var[:], mean[:], -1.0, mean[:],
                                           op0=Alu.mult, op1=Alu.mult)
            nc.vector.tensor_add(var[:], var[:], ex2[:])  # ex2 - mean^2
            nc.vector.tensor_scalar_add(var[:], var[:], EPS)
            std = sb.tile([2 * F, 1], f32)
            nc.scalar.activation(std[:], var[:], Act.Sqrt)
            rstd = sb.tile([2 * F, 1], f32)
            nc.vector.reciprocal(rstd[:], std[:])
            scale = sb.tile([2 * F, 1], f32)
            nc.vector.tensor_mul(scale[:], gb_sb[:, 0:1], rstd[:])
            nbias = sb.tile([2 * F, 1], f32)
            nc.vector.scalar_tensor_tensor(nbias[:], mean[:], -1.0, scale[:],
                                           op0=Alu.mult, op1=Alu.mult)
            nc.vector.tensor_add(nbias[:], nbias[:], gb_sb[:, 1:2])

            fin = sb.tile([P, R], f32)
            nc.scalar.activation(fin[:], hpT[:], Act.Identity,
                                 bias=nbias[:], scale=scale[:])
            finl = sb.tile([P, R], f32)
            nc.vector.scalar_tensor_tensor(finl[:], fin[:], ALPHA, fin[:],
                                           op0=Alu.mult, op1=Alu.max)

            for q in range(RT):
                tp = psB.tile([P, P], f32, tag="tmp")
                nc.tensor.transpose(tp[:], finl[:, P * q:P * (q + 1)],
                                    ident[:])
                ob = sbt.tile([P, P], f32, tag="ob")
                nc.vector.tensor_copy(ob[:], tp[:])
                nc.sync.dma_start(out_p[P * q:P * (q + 1), :], ob[:])

    nc.compile()
    return nc


def _get_nc():
    if "nc" not in _CACHED:
        _CACHED["nc"] = build_nc()
    return _CACHED["nc"]


def make_in_maps(h, adj, W1, W2, a, gamma, beta):
    h = np.asarray(h, dtype=np.float32)
    adj = np.asarray(adj, dtype=np.float32)
    W12 = np.concatenate([np.asarray(W1, np.float32),
                          np.asarray(W2, np.float32)], axis=1)
    a_flat = np.asarray(a, np.float32).reshape(2 * F)
    a_np = np.ascontiguousarray(np.stack([a_flat[:F], a_flat[F:]], axis=1))
    gb = np.stack([np.asarray(gamma, np.float32),
                   np.asarray(beta, np.float32)], axis=1)
    ident = np.eye(P, dtype=np.float32)

    adj_f8 = adj.astype(ml_dtypes.float8_e4m3fn)
    hTf = np.ascontiguousarray(h.T)

    in_maps = []
    for c in range(M_CORES):
        r0 = c * R
        sh = adj_f8[r0:r0 + R, :]
        dT = np.zeros((N, R), dtype=ml_dtypes.bfloat16)
        dT[np.arange(r0, r0 + R), np.arange(R)] = 1
        in_maps.append({
            "hT": np.ascontiguousarray(h[r0:r0 + R, :].T),
            "hTf": hTf,
            "adjb8": np.ascontiguousarray(sh),
            "adjbT": np.ascontiguousarray(sh.T),
            "dT": dT,
            "W12": W12,
            "a": a_np,
            "gb": gb,
            "ident": ident,
        })
    return in_maps


def kernel(h, adj, W1, W2, a, gamma, beta):
    from concourse.bass_utils import run_bass_kernel_spmd

    in_maps = make_in_maps(h, adj, W1, W2, a, gamma, beta)
    nc = _get_nc()
    res = run_bass_kernel_spmd(nc, in_maps, core_ids=list(range(M_CORES)))
    outs = [np.asarray(res.results[c]["out"]) for c in range(M_CORES)]
    return np.concatenate(outs, axis=0)



# revision 5
# speedup vs baseline: 1.6276x; 1.6276x over previous
"""Distributed Bass kernel for nn_AttentionLayer (2-branch GAT-style layer).

Row-shard over 8 NeuronCores (512 rows each). All per-row tensors kept in
"transposed" layout on chip (k on SBUF partitions, own-row i on free) so the
masked softmax feeds the PE attention matmuls without transposes:

  e_b^T[k, i] = prelu(s1_b[i] + s2_b[k])          (one ACT op, alpha=0.2)
  p = exp(e)  (bf16) ;  pt = p * mask01           (multiplicative masking)
  [acc_b; den_b] = [Wh_b | 1]^T @ pt              (fused numerator+denominator)

adj2^T is computed on PE in fp8 DoubleRow (exact: adj is 0/1, psum f32) from a
REPLICATED full-adj input laid out host-side in DoubleRow tile order -- no
AllGather, no collective barrier on the critical path (a dummy AllReduce at
t=0 absorbs the one-time cc-stream init). adj2's diagonal is zeroed by fusing
a per-core (1 - diag) fp8 mask into the min(cnt,1) STT. BatchNorm batch stats
via one tiny AllReduce at the end.
"""

import sys
import numpy as np

for _p in ("/opt/trn_rl_repo", "/opt/trn_rl_repo/concourse"):
    if _p not in sys.path:
        sys.path.insert(0, _p)

import ml_dtypes

N = 4096
M_CORES = 8
R = N // M_CORES          # 512 rows per core
IN_F = 512
HALF = IN_F // 2          # 256
F = 64
P = 128                   # partitions
NT = N // P               # 32 column (k) tiles
NG = 8                    # adj2 psum groups (4 kt each)
G = NT // NG              # 4 kt tiles per group
TP = 8                    # t-pair passes per group load (16 t passes = 8 pairs)
ALPHA = 0.2
EPS = 1e-5
INV_N = 1.0 / N

_CACHED = {}


def build_nc():
    from concourse import bacc, tile, mybir

    f32 = mybir.dt.float32
    bf16 = mybir.dt.bfloat16
    fp8 = mybir.dt.float8e4
    Alu = mybir.AluOpType
    Act = mybir.ActivationFunctionType
    DR = mybir.MatmulPerfMode.DoubleRow

    nc = bacc.Bacc("TRN2", target_bir_lowering=False, debug=False,
                   num_devices=M_CORES)

    hTs_p = nc.declare_dram_parameter("hTs", [P, 4, R], f32, isOutput=False)
    hTfb_p = nc.declare_dram_parameter("hTfb", [P, 4, N], bf16, isOutput=False)
    adjT_p = nc.declare_dram_parameter("adjT", [P, 16, 2, R], fp8,
                                       isOutput=False)
    adjDR_p = nc.declare_dram_parameter("adjDR", [NG, TP, P, 2, 2, R], fp8,
                                        isOutput=False)
    dinv_p = nc.declare_dram_parameter("dinv", [P, NT, R], fp8, isOutput=False)
    Wsb_p = nc.declare_dram_parameter("Wsb", [P, 2, 2 * F], f32,
                                      isOutput=False)
    a1c_p = nc.declare_dram_parameter("a1c", [P, 1], f32, isOutput=False)
    a2bc_p = nc.declare_dram_parameter("a2bc", [P, F], f32, isOutput=False)
    gb4_p = nc.declare_dram_parameter("gb4", [F, 4], f32, isOutput=False)
    out_p = nc.declare_dram_parameter("out", [P, R], f32, isOutput=True)

    RG = [list(range(M_CORES))]

    with tile.TileContext(nc) as tc:
        with (
            tc.tile_pool(name="sb", bufs=1) as sb,
            tc.tile_pool(name="af", bufs=8) as afp,
            tc.tile_pool(name="ep", bufs=4) as epool,
            tc.tile_pool(name="pp", bufs=4) as ppool,
            tc.tile_pool(name="mp", bufs=4) as mpool,
            tc.tile_pool(name="ptp", bufs=6) as ptpool,
            tc.tile_pool(name="pacc", bufs=1, space="PSUM") as pacc,
            tc.tile_pool(name="ptmp", bufs=6, space="PSUM") as ptmp,
            tc.tile_pool(name="dram", bufs=1, space="DRAM") as dram,
        ):
            # ---- dummy collective at t=0: absorbs the one-time cc-stream
            # barrier so the real stats AllReduce at the end is cheap.
            dumb_in = dram.tile([2, 1], f32)
            dumb_sb = sb.tile([2, 1], f32)
            nc.gpsimd.memset(dumb_sb[:], 0.0)
            nc.gpsimd.dma_start(dumb_in[:], dumb_sb[:])
            dumb_out = dram.tile([2, 1], f32, addr_space="Shared")
            nc.gpsimd.collective_compute(
                "AllReduce", Alu.add, replica_groups=RG,
                ins=[dumb_in[:].opt()], outs=[dumb_out[:].opt()])

            # ---- persistent loads ----
            # sync (HWDGE ring 1): adjT first, then the af stream (main loop)
            adjT = sb.tile([P, 16, 2, R], fp8)
            nc.sync.dma_start(adjT[:], adjT_p[:])
            # scalar (HWDGE ring 2): small stuff, own-shard h, full h
            Wsb = sb.tile([P, 2, 2 * F], f32)
            nc.scalar.dma_start(Wsb[:], Wsb_p[:])
            a1c = sb.tile([P, 1], f32)
            nc.scalar.dma_start(a1c[:], a1c_p[:])
            a2bc = sb.tile([P, F], f32)
            nc.scalar.dma_start(a2bc[:], a2bc_p[:])
            gb4 = sb.tile([F, 4], f32)
            nc.scalar.dma_start(gb4[:], gb4_p[:])
            hTs = sb.tile([P, 4, R], f32)
            nc.scalar.dma_start(hTs[:], hTs_p[:])
            hTfb = sb.tile([P, 4, N], bf16)
            nc.scalar.dma_start(hTfb[:], hTfb_p[:])
            # gpsimd (SWDGE): dinv
            dinv = sb.tile([P, NT, R], fp8)
            nc.gpsimd.dma_start(dinv[:], dinv_p[:])

            ones64 = sb.tile([P, F], f32)
            nc.vector.memset(ones64[:], 1.0)
            ones1 = sb.tile([1, P], f32)
            nc.vector.memset(ones1[:], 1.0)
            # bf16 copy of W for the natural-layout Wh matmuls (bf16 lhsT)
            Wsbb = sb.tile([P, 2, 2 * F], bf16)
            nc.vector.tensor_copy(Wsbb[:], Wsb[:])

            # ---- local Wh^T shard (f32): rows 0:64 b1, 64:128 b2 ----
            whT_ps = ptmp.tile([P, R], f32, tag="tmp")
            for b in range(2):
                for t in range(2):
                    nc.tensor.matmul(
                        whT_ps[F * b:F * (b + 1), :],
                        Wsb[:, t, F * b:F * (b + 1)],
                        hTs[:, 2 * b + t, :],
                        start=(t == 0), stop=(t == 1),
                    )
            whT_sb = sb.tile([P, R], f32)
            nc.vector.tensor_copy(whT_sb[:], whT_ps[:])

            # ---- s1 vectors + partition broadcast (f32-exact) ----
            # a1c rows 0:64 and 64:128 both hold a1, so each branch's matmul
            # reads operands at its own base partition (no shift needed).
            s1bc = sb.tile([P, 2, R], f32)
            for b in range(2):
                s1_ps = ptmp.tile([1, R], f32, tag="tmp", name=f"s1ps{b}")
                nc.tensor.matmul(s1_ps[:], a1c[F * b:F * (b + 1), :],
                                 whT_sb[F * b:F * (b + 1), :],
                                 start=True, stop=True)
                s1_sb = sb.tile([1, R], f32, tag=f"s1sb{b}")
                nc.vector.tensor_copy(s1_sb[:], s1_ps[:])
                bc_ps = ptmp.tile([P, R], f32, tag="tmp", name=f"s1bc{b}")
                nc.tensor.matmul(bc_ps[:], ones1[:], s1_sb[:],
                                 start=True, stop=True)
                nc.vector.tensor_copy(s1bc[:, b, :], bc_ps[:])

            # ---- psum accumulators: [0:64]=numerator, [64:65]=denominator
            acc = [pacc.tile([F + 1, R], f32, tag=f"acc{b}", name=f"acc{b}")
                   for b in range(2)]

            # ---- whf tiles (natural layout, bf16) + ones col + s2 ----
            # whf[:, kt, 0:64]=Wh1, [64:65]=1, [65:129]=Wh2, [129:130]=1
            whf = sb.tile([P, NT, 2 * F + 2], bf16)
            nc.vector.memset(whf[:, :, F:F + 1], 1.0)
            nc.vector.memset(whf[:, :, 2 * F + 1:2 * F + 2], 1.0)
            s2 = sb.tile([P, 2, NT], f32)

            def wh_tiles(kt):
                for b in range(2):
                    whn = ptmp.tile([P, F], f32, tag="tmp",
                                    name=f"whn{kt}_{b}")
                    for t in range(2):
                        nc.tensor.matmul(
                            whn[:],
                            hTfb[:, 2 * b + t, P * kt:P * (kt + 1)],
                            Wsbb[:, t, F * b:F * (b + 1)],
                            start=(t == 0), stop=(t == 1),
                        )
                    scr = epool.tile([P, F], f32, tag="e", name=f"s2s{kt}_{b}")
                    nc.vector.scalar_tensor_tensor(
                        scr[:], whn[:], 1.0, a2bc[:],
                        op0=Alu.mult, op1=Alu.mult,
                        accum_out=s2[:, b, kt:kt + 1])
                    off = (F + 1) * b
                    nc.vector.tensor_copy(whf[:, kt, off:off + F], whn[:])

            def softmax_tile(b, kt, pt_mask_fn):
                e = epool.tile([P, R], f32, tag="e")
                nc.scalar.activation(e[:], s1bc[:, b, :], Act.Prelu,
                                     bias=s2[:, b, kt:kt + 1], alpha=ALPHA)
                p = ppool.tile([P, R], bf16, tag="p")
                nc.scalar.activation(p[:], e[:], Act.Exp)
                pt = pt_mask_fn(p)
                off = (F + 1) * b
                nc.tensor.matmul(acc[b][:], whf[:, kt, off:off + F + 1],
                                 pt[:], start=(kt == 0), stop=(kt == NT - 1))

            def mask_b1(kt):
                def fn(p):
                    pt = ptpool.tile([P, R], bf16, tag="pt")
                    nc.gpsimd.tensor_tensor(pt[:], p[:],
                                            adjT[:, kt // 2, kt % 2, :],
                                            op=Alu.mult)
                    return pt
                return fn

            def mask_b2(kt, cnt):
                def fn(p):
                    m = mpool.tile([P, R], bf16, tag="m")
                    nc.vector.scalar_tensor_tensor(
                        m[:], cnt[:], 1.0, dinv[:, kt, :],
                        op0=Alu.min, op1=Alu.mult)
                    pt = ptpool.tile([P, R], bf16, tag="pt")
                    nc.gpsimd.tensor_tensor(pt[:], p[:], m[:], op=Alu.mult)
                    return pt
                return fn

            # ---- main loop: adj2 DoubleRow groups + interleaved softmax ----
            # PE issue order: g0 DRs | wh tiles | g1 DRs | att g0 | g2 DRs |
            # att g1 | ... so att matmuls trail one group behind the DR chain.
            pend = []          # deferred softmax work: (g, cnts)

            def run_group_softmax(g, cnts):
                for j in range(G):
                    kt = G * g + j
                    softmax_tile(1, kt, mask_b2(kt, cnts[j]))
                    softmax_tile(0, kt, mask_b1(kt))

            for g in range(NG):
                cnts = [ptmp.tile([P, R], f32, tag="tmp", name=f"cnt{g}_{j}")
                        for j in range(G)]
                for tp in range(TP):
                    af = afp.tile([P, 2, 2, R], fp8, tag="af")
                    nc.sync.dma_start(af[:], adjDR_p[g, tp])
                    for j in range(G):
                        for dt in range(2):
                            nc.tensor.matmul(
                                cnts[j][:],
                                af[:, dt, :, P * j:P * (j + 1)],
                                adjT[:, 2 * tp + dt, :, :],
                                perf_mode=DR,
                                start=(tp == 0 and dt == 0),
                                stop=(tp == TP - 1 and dt == 1),
                            )
                pend.append((g, cnts))
                if g == 0:
                    # Wh tiles: hTfb is resident by now; interleave after g0.
                    for kt in range(NT):
                        wh_tiles(kt)
                if g >= 1:
                    run_group_softmax(*pend.pop(0))
            while pend:
                run_group_softmax(*pend.pop(0))

            # ---- epilogue: normalize, BN stats AR, BN+lrelu, store ----
            stats_in = dram.tile([2 * F, 2], f32)
            hp = []
            for b in range(2):
                rct = sb.tile([F + 1, R], f32, tag=f"rct{b}")
                nc.vector.reciprocal(rct[F:F + 1, :], acc[b][F:F + 1, :])
                rbc_ps = ptmp.tile([F, R], f32, tag="tmp", name=f"rbc{b}")
                nc.tensor.matmul(rbc_ps[:], ones64[F:F + 1, :],
                                 rct[F:F + 1, :], start=True, stop=True)
                rbc = sb.tile([F, R], f32, tag=f"rbc{b}")
                nc.vector.tensor_copy(rbc[:], rbc_ps[:])
                hpb = sb.tile([F, R], f32, tag=f"hp{b}")
                nc.vector.tensor_mul(hpb[:], acc[b][0:F, :], rbc[:])
                hp.append(hpb)
                sx = sb.tile([F, 2], f32, tag=f"sx{b}")
                nc.vector.tensor_reduce(sx[:, 0:1], hpb[:],
                                        axis=mybir.AxisListType.X, op=Alu.add)
                scr = ppool.tile([P, R], bf16, tag="p", name=f"sq{b}")
                nc.scalar.activation(scr[0:F, :], hpb[:], Act.Square,
                                     accum_out=sx[:, 1:2])
                nc.gpsimd.dma_start(stats_in[F * b:F * (b + 1), :], sx[:])

            stats_out = dram.tile([2 * F, 2], f32, addr_space="Shared")
            nc.gpsimd.collective_compute(
                "AllReduce", Alu.add, replica_groups=RG,
                ins=[stats_in[:].opt()], outs=[stats_out[:].opt()])

            for b in range(2):
                gst = sb.tile([F, 2], f32, tag=f"gst{b}")
                nc.sync.dma_start(gst[:], stats_out[F * b:F * (b + 1), :])
                mean = sb.tile([F, 1], f32, tag=f"mean{b}")
                nc.scalar.mul(mean[:], gst[:, 0:1], INV_N)
                ex2 = sb.tile([F, 1], f32, tag=f"ex2{b}")
                nc.scalar.mul(ex2[:], gst[:, 1:2], INV_N)
                var = sb.tile([F, 1], f32, tag=f"var{b}")
                nc.vector.scalar_tensor_tensor(var[:], mean[:], -1.0, mean[:],
                                               op0=Alu.mult, op1=Alu.mult)
                nc.vector.tensor_add(var[:], var[:], ex2[:])
                nc.vector.tensor_scalar_add(var[:], var[:], EPS)
                std = sb.tile([F, 1], f32, tag=f"std{b}")
                nc.scalar.activation(std[:], var[:], Act.Sqrt)
                rstd = sb.tile([F, 1], f32, tag=f"rstd{b}")
                nc.vector.reciprocal(rstd[:], std[:])
                scale = sb.tile([F, 1], f32, tag=f"scale{b}")
                nc.vector.tensor_mul(scale[:], gb4[:, 2 * b:2 * b + 1],
                                     rstd[:])
                nbias = sb.tile([F, 1], f32, tag=f"nbias{b}")
                nc.vector.scalar_tensor_tensor(nbias[:], mean[:], -1.0,
                                               scale[:], op0=Alu.mult,
                                               op1=Alu.mult)
                nc.vector.tensor_add(nbias[:], nbias[:],
                                     gb4[:, 2 * b + 1:2 * b + 2])
                # BN affine + final leakyrelu fused in one activation
                fin = sb.tile([F, R], f32, tag=f"fin{b}")
                nc.scalar.activation(fin[:], hp[b][:], Act.Prelu,
                                     bias=nbias[:], scale=scale[:],
                                     alpha=ALPHA)
                nc.gpsimd.dma_start(out_p[F * b:F * (b + 1), :], fin[:])

    nc.compile()
    return nc


def _get_nc():
    if "nc" not in _CACHED:
        _CACHED["nc"] = build_nc()
    return _CACHED["nc"]


def make_in_maps(h, adj, W1, W2, a, gamma, beta):
    fp8 = ml_dtypes.float8_e4m3fn
    bf16 = ml_dtypes.bfloat16
    h = np.asarray(h, dtype=np.float32)
    adj = np.asarray(adj, dtype=np.float32)
    W1 = np.asarray(W1, np.float32)
    W2 = np.asarray(W2, np.float32)
    a_flat = np.asarray(a, np.float32).reshape(2 * F)
    gamma = np.asarray(gamma, np.float32)
    beta = np.asarray(beta, np.float32)

    adj8 = adj.astype(fp8)
    # adjDR[g, tp, p, dt, s, k] = adj[256*(2tp+dt) + 128*s + p, 512*g + k]
    t1 = adj8.reshape(TP, 2, 2, P, NG, R)          # [tp, dt, s, p, g, k]
    adjDR = np.ascontiguousarray(t1.transpose(4, 0, 3, 1, 2, 5))

    hT = h.T                                        # [IN_F, N]
    hTfb = np.ascontiguousarray(
        hT.astype(bf16).reshape(4, P, N).transpose(1, 0, 2))

    Wsb = np.ascontiguousarray(
        np.concatenate([W1, W2], axis=1).reshape(2, P, 2 * F)
        .transpose(1, 0, 2))
    a1c = np.ascontiguousarray(
        np.concatenate([a_flat[:F], a_flat[:F]]).reshape(P, 1))
    a2bc = np.ascontiguousarray(
        np.broadcast_to(a_flat[F:], (P, F)))
    gb4 = np.ascontiguousarray(
        np.stack([gamma[:F], beta[:F], gamma[F:], beta[F:]], axis=1))

    in_maps = []
    for c in range(M_CORES):
        r0 = c * R
        # adjT[p, t, s, i] = adj[r0+i, 256t+128s+p]
        ash = adj8[r0:r0 + R, :].T                  # [N(t_glob), R(i)]
        adjT = np.ascontiguousarray(
            ash.reshape(16, 2, P, R).transpose(2, 0, 1, 3))
        # dinv[p, kt, i] = 0 where 128*kt + p == r0 + i
        dinv = np.ones((P, NT, R), dtype=fp8)
        ii = np.arange(R)
        kk = r0 + ii
        dinv[kk % P, kk // P, ii] = 0
        hTs = np.ascontiguousarray(
            hT[:, r0:r0 + R].reshape(4, P, R).transpose(1, 0, 2))
        in_maps.append({
            "hTs": hTs,
            "hTfb": hTfb,
            "adjT": adjT,
            "adjDR": adjDR,
            "dinv": dinv,
            "Wsb": Wsb,
            "a1c": a1c,
            "a2bc": a2bc,
            "gb4": gb4,
        })
    return in_maps


def kernel(h, adj, W1, W2, a, gamma, beta):
    from concourse.bass_utils import run_bass_kernel_spmd

    in_maps = make_in_maps(h, adj, W1, W2, a, gamma, beta)
    nc = _get_nc()
    res = run_bass_kernel_spmd(nc, in_maps, core_ids=list(range(M_CORES)))
    out = np.empty((N, 2 * F), dtype=np.float32)
    for c in range(M_CORES):
        out[c * R:(c + 1) * R, :] = np.asarray(res.results[c]["out"]).T
    return out
